# revision 1
# baseline (speedup 1.0000x reference)
"""Trainium2 Bass kernel for the 13-branch scattering-GAT network.

Strategy (8 NeuronCores, row-parallel):
  - Nodes sharded 512/core. U and psi shards are host-transposed and cast to
    bf16; psi stays resident in SBUF so its two uses (level-1 and level-2
    wavelets) cost one HBM read.
  - Three small AllGathers carry |y1|, |y2| and h||es between phases.
  - Edge softmax-aggregation is done as one dense matmul against a host-built
    0/1 destination-indicator matrix (edges grouped into four 128-dst windows
    per core), which computes both sum(w*h) and z=sum(w) per destination.
    Self-loops are folded in locally without any gather.
"""

import sys

sys.path.insert(0, "/opt/trn_rl_repo")

import numpy as np
import ml_dtypes

import concourse.bass as bass
import concourse.mybir as mybir
import concourse.tile as tile
from concourse import bacc
from concourse.bass_utils import run_bass_kernel_spmd

R = 8          # cores
N = 4096       # nodes
S = N // R     # nodes per core (512)
F = 32         # features
H = 2          # heads
G = 13         # branches
GH = G * H     # 26
NHID = 64
C = 10
J = 3
KT = N // 128  # 32 contraction tiles
NW = S // 128  # 4 dst windows per core
HROW = 896     # padded AG row width (1792B, 256B-aligned)
EW = G * 66    # 858 edge-matmul output width per dst window
NEG = 0.2

BF = mybir.dt.bfloat16
F32 = mybir.dt.float32
I16 = mybir.dt.int16

_bf = lambda a: np.ascontiguousarray(a.astype(ml_dtypes.bfloat16))
_f32 = lambda a: np.ascontiguousarray(a.astype(np.float32))

_PROGRAM_CACHE = {}


def build_program(KMT, CHK):
    """KMT: k-tiles per dst window; CHK: k-tiles per gather/compute chunk."""
    TE = NW * KMT            # total edge k-tiles
    KE = TE * 128            # padded edge count
    nc = bacc.Bacc("TRN2", target_bir_lowering=False, debug=False, num_devices=R)

    # ---------------- I/O ----------------
    d_af = nc.dram_tensor("af", [N, F], BF, kind="ExternalInput")
    d_psiT = nc.dram_tensor("psiT", [N, J * S], BF, kind="ExternalInput")
    d_uT = nc.dram_tensor("uT", [N, S], BF, kind="ExternalInput")
    d_wcat = nc.dram_tensor("wcat", [F, G * 68], BF, kind="ExternalInput")
    d_bias = nc.dram_tensor("bias", [128, G * H * F], F32, kind="ExternalInput")
    d_mw = nc.dram_tensor("mw", [NHID, G * NHID], BF, kind="ExternalInput")
    d_mbp = nc.dram_tensor("mbp", [NHID, G], F32, kind="ExternalInput")
    d_outw = nc.dram_tensor("outw", [7 * 128, C], BF, kind="ExternalInput")
    d_ind = nc.dram_tensor("ind", [TE * 128, 128], BF, kind="ExternalInput")
    d_gidx = nc.dram_tensor("gidx", [128, KE // 16], I16, kind="ExternalInput")
    d_didx = nc.dram_tensor("didx", [128, KE // 16], I16, kind="ExternalInput")
    d_out = nc.dram_tensor("out", [S, C], F32, kind="ExternalOutput")

    from concourse.masks import make_identity

    with tile.TileContext(nc) as tc:
        with (
            tc.tile_pool(name="const", bufs=1) as kc,
            tc.tile_pool(name="work", bufs=1) as wk,
            tc.tile_pool(name="dram", bufs=1, space="DRAM") as dram,
        ):
            # ---------------- constant loads (prefetch early) ----------------
            ident = kc.tile([128, 128], BF)
            make_identity(nc, ident[:])
            identf = kc.tile([128, 128], F32)
            make_identity(nc, identf[:])

            af_sb = kc.tile([128, KT * F], BF)
            nc.sync.dma_start(
                af_sb[:].rearrange("p (k f) -> p k f", f=F),
                d_af[:].rearrange("(k p) f -> p k f", p=128),
            )
            wcat_sb = kc.tile([F, G * 68], BF)
            nc.sync.dma_start(wcat_sb[:], d_wcat[:])
            bias_sb = kc.tile([128, G * H * F], F32)
            nc.sync.dma_start(bias_sb[:], d_bias[:])
            mw_sb = kc.tile([NHID, G * NHID], BF)
            nc.sync.dma_start(mw_sb[:], d_mw[:])
            mbp_sb = kc.tile([NHID, G], F32)
            nc.sync.dma_start(mbp_sb[:], d_mbp[:])
            outw_sb = kc.tile([128, 7 * C], BF)
            nc.sync.dma_start(
                outw_sb[:].rearrange("p (t c) -> p t c", c=C),
                d_outw[:].rearrange("(t p) c -> p t c", p=128),
            )
            gidx_sb = kc.tile([128, KE // 16], I16)
            nc.sync.dma_start(gidx_sb[:], d_gidx[:])
            didx_sb = kc.tile([128, KE // 16], I16)
            nc.sync.dma_start(didx_sb[:], d_didx[:])
            ind_sb = kc.tile([128, TE * 128], BF)
            nc.sync.dma_start(
                ind_sb[:].rearrange("p (t c) -> p t c", c=128),
                d_ind[:].rearrange("(t p) c -> p t c", p=128),
            )
            u_sb = kc.tile([128, KT * S], BF)
            nc.sync.dma_start(
                u_sb[:].rearrange("p (k n) -> p k n", n=S),
                d_uT[:].rearrange("(k p) n -> p k n", p=128),
            )

            # DRAM bounce buffers
            aga1_in = dram.tile([S, J * F], BF)
            aga1_out = dram.tile([N, J * F], BF)
            aga2_in = dram.tile([S, J * J * F], BF)
            aga2_out = dram.tile([N, J * J * F], BF)
            agh_in = dram.tile([S, HROW], BF)
            agh_out = dram.tile([N, HROW], BF)
            ed_dram = dram.tile([S, 64], F32)
            rg = [list(range(R))]

            # ================= phases 2-3: wavelet tree =================
            with tc.tile_pool(name="p23", bufs=1) as p23:
                psi_sb = p23.tile([128, KT * J * S], BF)
                for kt4 in range(8):
                    nc.sync.dma_start(
                        psi_sb[:, kt4 * 4 * J * S:(kt4 + 1) * 4 * J * S]
                        .rearrange("p (k c) -> p k c", c=J * S),
                        d_psiT[kt4 * 512:(kt4 + 1) * 512, :]
                        .rearrange("(k p) c -> p k c", p=128),
                    )

                # ---- phase 2: y1_j = psi_j @ af
                a1T = p23.tile([F, J * S], BF)
                with tc.tile_pool(name="psA2", bufs=1, space="PSUM") as psA2:
                    for j in range(J):
                        p_y1 = psA2.tile([F, S], F32, tag="y1", bufs=2)
                        for kt in range(KT):
                            nc.tensor.matmul(
                                p_y1[:],
                                lhsT=af_sb[:, kt * F:(kt + 1) * F],
                                rhs=psi_sb[:, kt * J * S + j * S: kt * J * S + (j + 1) * S],
                                start=(kt == 0), stop=(kt == KT - 1),
                            )
                        nc.scalar.activation(a1T[:, j * S:(j + 1) * S], p_y1[:],
                                             mybir.ActivationFunctionType.Abs)
                    a1loc = p23.tile([128, NW * J * F], BF)
                    for j in range(J):
                        for mt in range(NW):
                            p_tp = psA2.tile([128, F], BF, tag="tp", bufs=2)
                            nc.tensor.transpose(
                                p_tp[:],
                                a1T[:, j * S + mt * 128: j * S + (mt + 1) * 128],
                                ident[:F, :F],
                            )
                            nc.vector.tensor_copy(
                                a1loc[:, mt * J * F + j * F: mt * J * F + (j + 1) * F],
                                p_tp[:],
                            )
                for mt in range(NW):
                    nc.sync.dma_start(
                        aga1_in[mt * 128:(mt + 1) * 128, :],
                        a1loc[:, mt * J * F:(mt + 1) * J * F],
                    )
                nc.gpsimd.collective_compute(
                    "AllGather", mybir.AluOpType.bypass, replica_groups=rg,
                    ins=[aga1_in[:].opt()], outs=[aga1_out[:].opt()],
                )
                a1_sb = p23.tile([128, KT * J * F], BF)
                nc.sync.dma_start(
                    a1_sb[:].rearrange("p (k c) -> p k c", c=J * F),
                    aga1_out[:].rearrange("(k p) c -> p k c", p=128),
                )

                # ---- phase 3: y2_{j,k} = psi_k @ a1_j
                a2T = p23.tile([J * F, J * S], BF)
                a2loc = p23.tile([128, NW * J * J * F], BF)
                with tc.tile_pool(name="psA3", bufs=1, space="PSUM") as psA3:
                    for k in range(J):
                        p_y2 = psA3.tile([J * F, S], F32, tag="y2", bufs=2)
                        for kt in range(KT):
                            nc.tensor.matmul(
                                p_y2[:],
                                lhsT=a1_sb[:, kt * J * F:(kt + 1) * J * F],
                                rhs=psi_sb[:, kt * J * S + k * S: kt * J * S + (k + 1) * S],
                                start=(kt == 0), stop=(kt == KT - 1),
                            )
                        nc.scalar.activation(a2T[:, k * S:(k + 1) * S], p_y2[:],
                                             mybir.ActivationFunctionType.Abs)
                    for k in range(J):
                        for mt in range(NW):
                            p_tp2 = psA3.tile([128, J * F], BF, tag="tp2", bufs=2)
                            nc.tensor.transpose(
                                p_tp2[:],
                                a2T[:, k * S + mt * 128: k * S + (mt + 1) * 128],
                                ident[:J * F, :J * F],
                            )
                            nc.vector.tensor_copy(
                                a2loc[:, mt * J * J * F + k * J * F:
                                      mt * J * J * F + (k + 1) * J * F],
                                p_tp2[:],
                            )
                for mt in range(NW):
                    nc.sync.dma_start(
                        aga2_in[mt * 128:(mt + 1) * 128, :],
                        a2loc[:, mt * J * J * F:(mt + 1) * J * J * F],
                    )
                nc.gpsimd.collective_compute(
                    "AllGather", mybir.AluOpType.bypass, replica_groups=rg,
                    ins=[aga2_in[:].opt()], outs=[aga2_out[:].opt()],
                )

            # ================= phases 4-5: coefs + GAT linear =================
            hes_sb = wk.tile([128, NW * HROW], BF)
            nc.vector.memset(
                hes_sb[:].rearrange("p (m c) -> p m c", c=HROW)[:, :, 858:HROW], 0.0)
            esloc = wk.tile([128, NW * GH], F32)
            edloc = wk.tile([128, NW * 64], F32)
            nc.vector.memset(edloc[:], 0.0)
            with tc.tile_pool(name="p45", bufs=1) as p45:
                acts = p45.tile([128, KT * G * F], BF)
                nc.vector.tensor_copy(
                    acts[:].rearrange("p (k c) -> p k c", c=G * F)[:, :, 0:F],
                    af_sb[:].rearrange("p (k f) -> p k f", f=F),
                )
                nc.sync.dma_start(
                    acts[:].rearrange("p (k c) -> p k c", c=G * F)[:, :, F:(1 + J) * F],
                    aga1_out[:].rearrange("(k p) c -> p k c", p=128),
                )
                for j in range(J):
                    for k in range(J):
                        # branch g = 4 + 3j + k  <- a2 col k*96 + j*32 + f
                        g = 4 + 3 * j + k
                        nc.sync.dma_start(
                            acts[:].rearrange("p (kk c) -> p kk c", c=G * F)
                            [:, :, g * F:(g + 1) * F],
                            aga2_out[:].rearrange("(t p) c -> p t c", p=128)
                            [:, :, (k * J + j) * F:(k * J + j + 1) * F],
                        )
                coefsT2 = p45.tile([F, G * S], BF)
                with tc.tile_pool(name="psB", bufs=1, space="PSUM") as psB:
                    for mg in range(4):
                        nb = 4 if mg < 3 else 1
                        p_c = psB.tile([128, S], F32, tag="coef", bufs=2)
                        for kt in range(KT):
                            nc.tensor.matmul(
                                p_c[: nb * F, :],
                                lhsT=acts[:, kt * G * F + mg * 4 * F:
                                          kt * G * F + (mg * 4 + nb) * F],
                                rhs=u_sb[:, kt * S:(kt + 1) * S],
                                start=(kt == 0), stop=(kt == KT - 1),
                            )
                        for gg in range(nb):
                            nc.vector.tensor_copy(
                                coefsT2[:, (mg * 4 + gg) * S:(mg * 4 + gg + 1) * S],
                                p_c[gg * F:(gg + 1) * F, :],
                            )
                    # ---- phase 5  (psum col: g<7 at g*68, g>=7 at 512+(g-7)*68
                    #                to keep each matmul inside one psum bank)
                    for mt in range(NW):
                        p_h = psB.tile([128, 1024], F32, tag="h5", bufs=2)
                        for g in range(G):
                            pc = g * 68 if g < 7 else 512 + (g - 7) * 68
                            nc.tensor.matmul(
                                p_h[:, pc:pc + 68],
                                lhsT=coefsT2[:, g * S + mt * 128: g * S + (mt + 1) * 128],
                                rhs=wcat_sb[:, g * 68:(g + 1) * 68],
                                start=True, stop=True,
                            )
                        for g0, ng in ((0, 7), (7, 6)):
                            base = 0 if g0 == 0 else 512
                            ph = p_h[:, base: base + ng * 68]\
                                .rearrange("p (g c) -> p g c", c=68)
                            nc.vector.tensor_copy(
                                hes_sb[:, mt * HROW + g0 * H * F:
                                       mt * HROW + (g0 + ng) * H * F]
                                .rearrange("p (g c) -> p g c", c=H * F),
                                ph[:, :, 0:H * F],
                            )
                            nc.vector.tensor_copy(
                                hes_sb[:, mt * HROW + G * H * F + g0 * H:
                                       mt * HROW + G * H * F + (g0 + ng) * H]
                                .rearrange("p (g c) -> p g c", c=H),
                                ph[:, :, H * F:H * F + H],
                            )
                            nc.vector.tensor_copy(
                                esloc[:, mt * GH + g0 * H: mt * GH + (g0 + ng) * H]
                                .rearrange("p (g c) -> p g c", c=H),
                                ph[:, :, H * F:H * F + H],
                            )
                            nc.vector.tensor_copy(
                                edloc[:, mt * 64 + g0 * H: mt * 64 + (g0 + ng) * H]
                                .rearrange("p (g c) -> p g c", c=H),
                                ph[:, :, H * F + H:H * F + 2 * H],
                            )
            for mt in range(NW):
                nc.sync.dma_start(
                    agh_in[mt * 128:(mt + 1) * 128, :],
                    hes_sb[:, mt * HROW:(mt + 1) * HROW],
                )
            nc.gpsimd.collective_compute(
                "AllGather", mybir.AluOpType.bypass, replica_groups=rg,
                ins=[agh_in[:].opt()], outs=[agh_out[:].opt()],
            )
            nc.sync.dma_start(
                ed_dram[:].rearrange("(mt p) c -> p mt c", p=128),
                edloc[:].rearrange("p (mt c) -> p mt c", c=64),
            )

            # ================= edge phase =================
            elu_s = wk.tile([128, NW * G * H * F], BF)
            with tc.tile_pool(name="pe", bufs=1) as pe, \
                 tc.tile_pool(name="psC", bufs=1, space="PSUM") as psC:
                ew_ps = [psC.tile([128, EW], F32, tag=f"ew{w}", bufs=1,
                                  name=f"ew{w}") for w in range(NW)]
                for w in range(NW):
                    done_k = 0
                    while done_k < KMT:
                        nk = min(CHK, KMT - done_k)
                        kt0 = w * KMT + done_k
                        hg = pe.tile([128, CHK * HROW], BF, tag="hg", bufs=2)
                        nc.gpsimd.dma_gather(
                            out_ap=hg[:, : nk * HROW].rearrange("p (c x) -> p c x", x=HROW),
                            in_ap=agh_out[:],
                            idxs_ap=gidx_sb[:, kt0 * 8:(kt0 + nk) * 8],
                            num_idxs=nk * 128,
                            num_idxs_reg=nk * 128,
                            elem_size=HROW,
                        )
                        edc = pe.tile([128, CHK * 128], BF, tag="edc", bufs=2)
                        nc.gpsimd.dma_gather(
                            out_ap=edc[:, : nk * 128].rearrange("p (c x) -> p c x", x=128),
                            in_ap=ed_dram[:].bitcast(BF),
                            idxs_ap=didx_sb[:, kt0 * 8:(kt0 + nk) * 8],
                            num_idxs=nk * 128,
                            num_idxs_reg=nk * 128,
                            elem_size=128,
                        )
                        wv = pe.tile([128, CHK * GH], F32, tag="wv", bufs=2)
                        wv3 = wv[:, : nk * GH].rearrange("p (c g) -> p c g", g=GH)
                        nc.vector.tensor_tensor(
                            out=wv3,
                            in0=hg[:, : nk * HROW].rearrange("p (c x) -> p c x", x=HROW)
                            [:, :, G * H * F: G * H * F + GH],
                            in1=edc[:, : nk * 128].bitcast(F32)
                            .rearrange("p (c x) -> p c x", x=64)[:, :, 0:GH],
                            op=mybir.AluOpType.add,
                        )
                        nc.vector.scalar_tensor_tensor(
                            out=wv3, in0=wv3, scalar=NEG, in1=wv3,
                            op0=mybir.AluOpType.mult, op1=mybir.AluOpType.max,
                        )
                        nc.scalar.activation(wv3, wv3, mybir.ActivationFunctionType.Exp)
                        rhs = pe.tile([128, CHK * EW], BF, tag="rhs", bufs=2)
                        rhs4 = rhs[:, : nk * EW].rearrange(
                            "p (c g q) -> p c g q", g=GH, q=33)
                        nc.vector.tensor_tensor(
                            out=rhs4[:, :, :, 0:F],
                            in0=hg[:, : nk * HROW].rearrange(
                                "p (c g f) -> p c g f", g=28, f=F)[:, :, 0:GH, :],
                            in1=wv3.to_broadcast([128, nk, GH, F]),
                            op=mybir.AluOpType.mult,
                        )
                        nc.vector.tensor_copy(rhs4[:, :, :, 32:33],
                                              wv3.to_broadcast([128, nk, GH, 1]))
                        for ck in range(nk):
                            kt = kt0 + ck
                            first = (done_k + ck == 0)
                            last = (done_k + ck == KMT - 1)
                            nc.tensor.matmul(
                                ew_ps[w][:, 0:512],
                                lhsT=ind_sb[:, kt * 128:(kt + 1) * 128],
                                rhs=rhs[:, ck * EW: ck * EW + 512],
                                start=first, stop=last, skip_group_check=True,
                            )
                            nc.tensor.matmul(
                                ew_ps[w][:, 512:EW],
                                lhsT=ind_sb[:, kt * 128:(kt + 1) * 128],
                                rhs=rhs[:, ck * EW + 512:(ck + 1) * EW],
                                start=first, stop=last, skip_group_check=True,
                            )
                        done_k += nk

                # ---- self loops + normalize + ELU(+1)
                for mt in range(NW):
                    wself = pe.tile([128, GH], F32, tag="wself", bufs=1)
                    nc.vector.tensor_tensor(
                        out=wself[:], in0=esloc[:, mt * GH:(mt + 1) * GH],
                        in1=edloc[:, mt * 64: mt * 64 + GH], op=mybir.AluOpType.add)
                    nc.vector.scalar_tensor_tensor(
                        out=wself[:], in0=wself[:], scalar=NEG, in1=wself[:],
                        op0=mybir.AluOpType.mult, op1=mybir.AluOpType.max)
                    nc.scalar.activation(wself[:], wself[:],
                                         mybir.ActivationFunctionType.Exp)
                    pse = ew_ps[mt][:].rearrange("p (g q) -> p g q", q=33)
                    nc.vector.tensor_tensor(
                        out=pse[:, :, 32:33], in0=pse[:, :, 32:33],
                        in1=wself[:].rearrange("p (g o) -> p g o", o=1),
                        op=mybir.AluOpType.add)
                    tmp = pe.tile([128, G * H * F], F32, tag="selfh", bufs=1)
                    nc.vector.tensor_tensor(
                        out=tmp[:].rearrange("p (g f) -> p g f", f=F),
                        in0=hes_sb[:, mt * HROW: mt * HROW + G * H * F]
                        .rearrange("p (g f) -> p g f", f=F),
                        in1=wself[:].rearrange("p (g o) -> p g o", o=1)
                        .to_broadcast([128, GH, F]),
                        op=mybir.AluOpType.mult)
                    nc.vector.tensor_tensor(
                        out=pse[:, :, 0:F], in0=pse[:, :, 0:F],
                        in1=tmp[:].rearrange("p (g f) -> p g f", f=F),
                        op=mybir.AluOpType.add)
                    zv = pe.tile([128, GH], F32, tag="zv", bufs=1)
                    nc.vector.tensor_copy(zv[:].rearrange("p (g o) -> p g o", o=1),
                                          pse[:, :, 32:33])
                    nc.vector.reciprocal(zv[:], zv[:])
                    t1 = pe.tile([128, G * H * F], F32, tag="t1", bufs=1)
                    nc.vector.tensor_tensor(
                        out=t1[:].rearrange("p (g f) -> p g f", f=F),
                        in0=pse[:, :, 0:F],
                        in1=zv[:].rearrange("p (g o) -> p g o", o=1)
                        .to_broadcast([128, GH, F]),
                        op=mybir.AluOpType.mult)
                    nc.vector.tensor_add(t1[:], t1[:], bias_sb[:])
                    t2 = pe.tile([128, G * H * F], F32, tag="t2", bufs=1)
                    nc.vector.tensor_scalar_min(t2[:], t1[:], 0.0)
                    nc.scalar.activation(t2[:], t2[:],
                                         mybir.ActivationFunctionType.Exp)
                    nc.vector.scalar_tensor_tensor(
                        out=elu_s[:, mt * G * H * F:(mt + 1) * G * H * F],
                        in0=t1[:], scalar=0.0, in1=t2[:],
                        op0=mybir.AluOpType.max, op1=mybir.AluOpType.add)

            # ================= MLP + head + log_softmax =================
            with tc.tile_pool(name="pf", bufs=1) as pf, \
                 tc.tile_pool(name="psD", bufs=1, space="PSUM") as psD:
                s1T = pf.tile([NHID, G * S], BF)
                for g in range(G):
                    for mt in range(NW):
                        p_t3 = psD.tile([NHID, 128], BF, tag="tp3", bufs=2)
                        nc.tensor.transpose(
                            p_t3[:],
                            elu_s[:, mt * G * H * F + g * H * F:
                                  mt * G * H * F + (g + 1) * H * F],
                            ident[:, :],
                        )
                        nc.vector.tensor_copy(
                            s1T[:, g * S + mt * 128: g * S + (mt + 1) * 128], p_t3[:])
                s2 = pf.tile([128, 7 * S], BF)
                nc.vector.memset(s2[64:128, 6 * S:7 * S], 0.0)
                nc.vector.memset(s2[64:65, 6 * S:7 * S], 1.0)
                for g in range(G):
                    p_m = psD.tile([NHID, S], F32, tag="mlp", bufs=2)
                    nc.tensor.matmul(
                        p_m[:], lhsT=mw_sb[:, g * NHID:(g + 1) * NHID],
                        rhs=s1T[:, g * S:(g + 1) * S], start=True, stop=True)
                    yb = pf.tile([NHID, S], F32, tag="yb", bufs=2)
                    nc.vector.tensor_scalar_add(yb[:], p_m[:], mbp_sb[:, g:g + 1])
                    ym = pf.tile([NHID, S], F32, tag="ym", bufs=2)
                    nc.vector.tensor_scalar_min(ym[:], yb[:], 0.0)
                    nc.scalar.activation(ym[:], ym[:],
                                         mybir.ActivationFunctionType.Exp)
                    nc.vector.scalar_tensor_tensor(
                        out=s2[(g % 2) * NHID:(g % 2 + 1) * NHID,
                               (g // 2) * S:(g // 2 + 1) * S],
                        in0=yb[:], scalar=0.0, in1=ym[:],
                        op0=mybir.AluOpType.max, op1=mybir.AluOpType.add)
                p_f = psD.tile([C, S], F32, tag="fin", bufs=1)
                for t in range(7):
                    nc.tensor.matmul(
                        p_f[:], lhsT=outw_sb[:, t * C:(t + 1) * C],
                        rhs=s2[:, t * S:(t + 1) * S],
                        start=(t == 0), stop=(t == 6))
                lg = pf.tile([C, S], F32)
                nc.vector.tensor_copy(lg[:], p_f[:])
                for mt in range(NW):
                    p_l = psD.tile([128, C], F32, tag="lsm", bufs=2)
                    nc.tensor.transpose(p_l[:], lg[:, mt * 128:(mt + 1) * 128],
                                        identf[:C, :C])
                    lt = pf.tile([128, C], F32, tag="lt", bufs=2)
                    mx = pf.tile([128, 1], F32, tag="mx", bufs=2)
                    nc.vector.reduce_max(mx[:], p_l[:], axis=mybir.AxisListType.X)
                    nc.vector.tensor_scalar_sub(lt[:], p_l[:], mx[:])
                    ex = pf.tile([128, C], F32, tag="ex", bufs=2)
                    nc.scalar.activation(ex[:], lt[:],
                                         mybir.ActivationFunctionType.Exp)
                    se = pf.tile([128, 1], F32, tag="se", bufs=2)
                    nc.vector.reduce_sum(se[:], ex[:], axis=mybir.AxisListType.X)
                    nc.scalar.activation(se[:], se[:],
                                         mybir.ActivationFunctionType.Ln)
                    oo = pf.tile([128, C], F32, tag="oo", bufs=2)
                    nc.vector.tensor_scalar_sub(oo[:], lt[:], se[:])
                    nc.sync.dma_start(d_out[mt * 128:(mt + 1) * 128, :], oo[:])

    nc.compile()
    return nc


def _host_prep(inputs):
    """Shard/transpose/cast inputs; build edge structures."""
    x = np.asarray(inputs["x"], np.float32)
    edge_index = np.asarray(inputs["edge_index"]).astype(np.int64)
    U = np.asarray(inputs["U"], np.float32)
    psi = np.asarray(inputs["psi"], np.float32)
    gat_W = np.asarray(inputs["gat_W"], np.float32)
    att_src = np.asarray(inputs["att_src"], np.float32)
    att_dst = np.asarray(inputs["att_dst"], np.float32)
    gat_b = np.asarray(inputs["gat_b"], np.float32)
    mlp_W = np.asarray(inputs["mlp_W"], np.float32)
    mlp_b = np.asarray(inputs["mlp_b"], np.float32)
    out_W = np.asarray(inputs["out_W"], np.float32)
    out_b = np.asarray(inputs["out_b"], np.float32)

    src, dst = edge_index[0], edge_index[1]
    E = src.shape[0]

    order = np.argsort(dst * 1, kind="stable")
    core_all = dst // S
    win_all = (dst % S) // 128
    key = core_all * NW + win_all
    order = np.argsort(key, kind="stable")
    counts = np.bincount(key, minlength=R * NW)
    maxw = counts.max()
    KMT = int((maxw + 127) // 128)
    KE = NW * KMT * 128
    TE = NW * KMT
    CHK = 6

    # shared weight packs
    wcat = np.zeros((F, G * 68), np.float32)
    for g in range(G):
        Wg = gat_W[g]                                   # [F, H*F]
        Wh = Wg.reshape(F, H, F)
        Ws = np.einsum("ihf,hf->ih", Wh, att_src[g])    # [F, H]
        Wd = np.einsum("ihf,hf->ih", Wh, att_dst[g])    # [F, H]
        wcat[:, g * 68: g * 68 + H * F] = Wg
        wcat[:, g * 68 + H * F: g * 68 + H * F + H] = Ws
        wcat[:, g * 68 + H * F + H: g * 68 + 68] = Wd
    bias = np.tile(gat_b.reshape(1, G * H * F), (128, 1)).astype(np.float32)
    mw = np.concatenate([mlp_W[g] for g in range(G)], axis=1)
    mbp = np.stack([mlp_b[g] - mlp_W[g].sum(0) for g in range(G)], 1)
    outw = np.zeros((7 * 128, C), np.float32)
    outw[:G * NHID, :] = out_W
    outw[G * NHID, :] = out_b - out_W.sum(0)

    af = np.abs(x)

    def wrap_idx(arr):
        a = arr.reshape(-1, 16).T.astype(np.int16)
        return np.ascontiguousarray(np.tile(a, (8, 1)))

    starts = np.zeros(R * NW + 1, np.int64)
    starts[1:] = np.cumsum(counts)
    sorted_e = order

    in_maps = []
    for r in range(R):
        sl = slice(r * S, (r + 1) * S)
        psiT = np.ascontiguousarray(
            psi[:, sl, :].transpose(2, 0, 1).reshape(N, J * S))
        uT = np.ascontiguousarray(U[sl, :].T)

        gsrc = np.zeros(KE, np.int64)
        ldst = np.zeros(KE, np.int64)
        valid = np.zeros(KE, bool)
        for w in range(NW):
            k = r * NW + w
            es = sorted_e[starts[k]:starts[k + 1]]
            base = w * KMT * 128
            gsrc[base: base + len(es)] = src[es]
            ldst[base: base + len(es)] = dst[es] - r * S
            valid[base: base + len(es)] = True
        ind = np.zeros((TE * 128, 128), np.float32)
        t_of = np.arange(KE) // 128
        w_of = t_of // KMT
        rows = np.arange(KE)[valid]
        ind[rows, (ldst - 128 * w_of)[valid]] = 1.0

        in_maps.append({
            "af": _bf(af),
            "psiT": _bf(psiT),
            "uT": _bf(uT),
            "wcat": _bf(wcat),
            "bias": bias,
            "mw": _bf(mw),
            "mbp": _f32(mbp),
            "outw": _bf(outw),
            "ind": _bf(ind),
            "gidx": wrap_idx(gsrc),
            "didx": wrap_idx(ldst),
        })
    return in_maps, KMT, CHK


def kernel(**inputs) -> np.ndarray:
    in_maps, KMT, CHK = _host_prep(inputs)
    key = (KMT, CHK)
    if key not in _PROGRAM_CACHE:
        _PROGRAM_CACHE[key] = build_program(KMT, CHK)
    nc = _PROGRAM_CACHE[key]
    res = run_bass_kernel_spmd(nc, in_maps, list(range(R)))
    out = np.concatenate([res.results[i]["out"] for i in range(R)], axis=0)
    return out.astype(np.float32)

